# revision 5
# baseline (speedup 1.0000x reference)
"""GlobalPointer-style head (RoPE'd QK^T with pad + strict-lower-tri masks).

Self-contained Trainium2 Bass kernel. Accepts FULL inputs, shards batch 16 ->
8 cores (2 per core), runs one SPMD Bass program, gathers FULL output.
HBM-bandwidth-bound (~28.4 MB/core at ~94% of the 358 GB/s per-core limit).

Structure (per batch, per core):
  - Masked-prefix constants prepared FIRST (pad-prep + prefix_fill) so the
    ramp window does useful work; x cast-loaded via SWDGE DMA (f32->bf16),
    chunks in REVERSED order (chunk 3 first) so early heads' chunk-3 work
    can start after one load+RoPE instead of four.
  - RoPE on DVE with small per-head-block tables [M, 128] broadcast over
    heads via stride-0 views (free on DVE): broadcast mul + 2 stride-2
    half-muls (pair swap) + add; k-half scaled by pad per-partition.
  - Per (head, chunk 3->0): PE transposes q^T/k^T into a 65-partition qkt
    tile whose partition 64 holds [ones | colb] bias rows; ONE K=65 matmul
    per out-chunk computes q'k' + padbias over LIVE columns (n >= c*128)
    only, at the same PE cost as K=64 (stream-bound).
  - Epilogue: DVE fused add (psum + tdiag) on the diag block, ACT/DVE
    copies for the live suffix. Output: first 3 heads + last head use
    per-chunk DMAs (start output traffic during the ramp / shorten the
    drain); all others ONE full-width 1 MB DMA (2 KB descriptor rows).
  - Masked prefix (exact f32 -V8*(2-pad)) pre-filled into the 3 rotating
    osb buffers once per batch per buffer; colfull built by a K=1 f32 PE
    matmul (gpsimd.partition_broadcast queues behind SWDGE DGE -> stalls).
"""

import sys

import numpy as np

for _p in ("/opt/trn_rl_repo",):
    if _p not in sys.path:
        sys.path.insert(0, _p)

import ml_dtypes  # noqa: E402

import concourse.mybir as mybir  # noqa: E402
import concourse.tile as tile  # noqa: E402
from concourse import bacc  # noqa: E402
from concourse.bass_utils import run_bass_kernel_spmd  # noqa: E402
from concourse.masks import make_identity  # noqa: E402

F32 = mybir.dt.float32
BF16 = mybir.dt.bfloat16

N_CORES = 8
B, M, H, D = 16, 512, 12, 64
BS = B // N_CORES  # batches per core
MC = M // 128  # m-chunks of 128
FW = H * 2 * D  # 1536 features per row
NEG = np.float32(1.0e12)
V8 = np.float32(NEG / np.float32(8.0))  # 1.25e11-ish, exact in fp32

LW = [M - 128 * c for c in range(MC)]  # live width per chunk: 512..128


def _tables():
    """Host-precomputed constants (functions of position only)."""
    pos = np.arange(M, dtype=np.float32)[:, None]
    inv_freq = np.power(
        np.float32(10000.0),
        (np.float32(-2.0) * np.arange(D // 2, dtype=np.float32) / np.float32(D)),
    )
    ang = pos * inv_freq[None, :]  # (M, 32)
    cos = np.repeat(np.cos(ang), 2, axis=1).astype(np.float32)  # (M, 64)
    sin = np.repeat(np.sin(ang), 2, axis=1).astype(np.float32)
    sign = np.where(np.arange(D) % 2 == 0, np.float32(-1.0), np.float32(1.0))
    sin_s = sin * sign[None, :]

    scale = np.float32(1.0 / 8.0)
    # per-head block: [q-feats (scaled 1/8) | k-feats]; broadcast over heads
    # on-device via stride-0 views (replication in SBUF is unnecessary)
    cos_full = np.concatenate([cos * scale, cos], axis=1).astype(
        ml_dtypes.bfloat16
    )  # (M, 128)
    sin_full = np.concatenate([sin_s * scale, sin_s], axis=1).astype(
        ml_dtypes.bfloat16
    )

    # strict-lower 128x128 diagonal block, fp32
    p = np.arange(128)
    tdiag = np.where(p[:, None] > p[None, :], -V8, np.float32(0.0)).astype(np.float32)

    return cos_full, sin_full, tdiag


def build_nc():
    nc = bacc.Bacc("TRN2", target_bir_lowering=False, debug=False)

    x_d = nc.dram_tensor("x", [BS, M, FW], F32, kind="ExternalInput")
    mask_d = nc.dram_tensor("mask", [BS, M], F32, kind="ExternalInput")
    cos_d = nc.dram_tensor("cos_t", [M, 2 * D], BF16, kind="ExternalInput")
    sin_d = nc.dram_tensor("sin_t", [M, 2 * D], BF16, kind="ExternalInput")
    tdiag_d = nc.dram_tensor("tdiag", [128, 128], F32, kind="ExternalInput")
    out_d = nc.dram_tensor("out", [BS, H, M, M], F32, kind="ExternalOutput")

    mult = mybir.AluOpType.mult
    add = mybir.AluOpType.add

    with tile.TileContext(nc) as tc:
        with (
            tc.tile_pool(name="const", bufs=1) as cpool,
            tc.tile_pool(name="xin", bufs=1) as xpool,
            tc.tile_pool(name="rope", bufs=1) as rpool,
            tc.tile_pool(name="small", bufs=2) as spool,
            tc.tile_pool(name="ps_t", bufs=3, space="PSUM") as pst_pool,
            tc.tile_pool(name="ps_mm", bufs=3, space="PSUM") as psm_pool,
            tc.tile_pool(name="ps_cf", bufs=1, space="PSUM") as pcf_pool,
        ):
            ones_f32 = cpool.tile([1, 128], F32)
            nc.vector.memset(ones_f32[:], 1.0)

            # RoPE tables (one head-block wide), rows (c p) -> [128, (c f)]
            cos_sb = cpool.tile([128, MC * 2 * D], BF16)
            nc.sync.dma_start(
                out=cos_sb[:].rearrange("p (c f) -> p c f", c=MC),
                in_=cos_d[:].rearrange("(c p) f -> p c f", p=128),
            )
            sin_sb = cpool.tile([128, MC * 2 * D], BF16)
            nc.sync.dma_start(
                out=sin_sb[:].rearrange("p (c f) -> p c f", c=MC),
                in_=sin_d[:].rearrange("(c p) f -> p c f", p=128),
            )
            tdiag_sb = cpool.tile([128, 128], F32)
            nc.sync.dma_start(out=tdiag_sb[:], in_=tdiag_d[:])

            # persistent qkt/osb buffers, rotated manually so per-batch
            # constants (bias row, masked-prefix columns) are written once
            # per batch per buffer, not once per head
            QBUFS = 3
            qkt_bufs = [
                cpool.tile([65, MC * 256], BF16, name=f"qkt{i}")
                for i in range(QBUFS)
            ]
            ident = cpool.tile([128, 128], BF16, name="ident")
            OBUFS = 3
            osb_bufs = [
                cpool.tile([128, MC * M], F32, name=f"osb{i}")
                for i in range(OBUFS)
            ]

            state = {}

            def pad_prep(b):
                padrow = spool.tile([1, M], F32, tag="padrow", name=f"pr{b}")
                nc.sync.dma_start(out=padrow[:], in_=mask_d[b : b + 1, :])
                colb = spool.tile([1, M], BF16, tag="colb", name=f"cb{b}")
                nc.vector.tensor_scalar(
                    out=colb[:], in0=padrow[:], scalar1=float(V8),
                    scalar2=float(-V8), op0=mult, op1=add,
                )
                rowvals = spool.tile([1, M], F32, tag="rowvals", name=f"rv{b}")
                nc.vector.tensor_scalar(
                    out=rowvals[:], in0=padrow[:], scalar1=float(V8),
                    scalar2=float(-2.0 * V8), op0=mult, op1=add,
                )
                ps_cf = pcf_pool.tile([128, M], F32, tag="pscf", bufs=1,
                                      name=f"pscf{b}")
                nc.tensor.matmul(
                    ps_cf[:], ones_f32[:], rowvals[:], start=True, stop=True
                )
                colfull = spool.tile([128, M], F32, tag="colfull", name=f"cf{b}")
                nc.vector.tensor_copy(out=colfull[:], in_=ps_cf[:])
                padcol = spool.tile([128, MC], F32, tag="padcol", name=f"pc{b}")
                nc.sync.dma_start(
                    out=padcol[:],
                    in_=mask_d[b, :].rearrange("(c p) -> p c", p=128),
                )
                state[b] = {"colb": colb, "colfull": colfull, "padcol": padcol}

            def prefix_fill(b):
                colfull = state[b]["colfull"]
                for i in range(OBUFS):
                    for c in range(1, MC):
                        pw = c * 128
                        dst = osb_bufs[i][:, c * M : c * M + pw]
                        if (i + c) % 2 == 0:
                            nc.scalar.copy(out=dst, in_=colfull[:, 0:pw])
                        else:
                            nc.vector.tensor_copy(out=dst, in_=colfull[:, 0:pw])

            pad_prep(0)
            prefix_fill(0)

            copy_rr = 0
            for b in range(BS):
                if b > 0:
                    pad_prep(b)
                    prefix_fill(b)
                # ---- load x chunks (reversed: chunk 3 first), SWDGE cast
                xb = [None] * MC
                for c in reversed(range(MC)):
                    t = xpool.tile(
                        [128, FW], BF16, tag="xb", bufs=4, name=f"xb{b}{c}"
                    )
                    nc.gpsimd.dma_start(
                        out=t[:], in_=x_d[b, c * 128 : (c + 1) * 128, :]
                    )
                    xb[c] = t
                if b == 0:
                    make_identity(nc, ident)

                colb = state[b]["colb"]
                padcol = state[b]["padcol"]

                # ---- RoPE, full-width flat tables
                xr = [None] * MC
                for c in reversed(range(MC)):
                    src = xb[c]
                    cs = slice(c * 2 * D, (c + 1) * 2 * D)
                    xr_c = rpool.tile([128, FW], BF16, tag="xr", bufs=8)
                    t1 = rpool.tile([128, FW], BF16, tag="t1", bufs=2)
                    # t1 = pairswap(src) * sin (two stride-2 half muls,
                    # sin table broadcast over heads via stride-0 view)
                    s4 = src[:].rearrange("p (h a two) -> p h a two", two=2, a=D)
                    t14 = t1[:].rearrange("p (h a two) -> p h a two", two=2, a=D)
                    n4 = sin_sb[:, cs].rearrange(
                        "p (o a two) -> p o a two", o=1, two=2
                    )
                    nc.vector.tensor_mul(
                        out=t14[:, :, :, 0],
                        in0=s4[:, :, :, 1],
                        in1=n4[:, :, :, 0].to_broadcast((128, H, D)),
                    )
                    nc.vector.tensor_mul(
                        out=t14[:, :, :, 1],
                        in0=s4[:, :, :, 0],
                        in1=n4[:, :, :, 1].to_broadcast((128, H, D)),
                    )
                    # xr = src * cos + t1 (cos broadcast over heads)
                    s3 = src[:].rearrange("p (h f) -> p h f", f=2 * D)
                    x3 = xr_c[:].rearrange("p (h f) -> p h f", f=2 * D)
                    nc.vector.tensor_mul(
                        out=x3,
                        in0=s3,
                        in1=cos_sb[:, cs].unsqueeze(1).to_broadcast(
                            (128, H, 2 * D)
                        ),
                    )
                    nc.vector.tensor_add(out=xr_c[:], in0=xr_c[:], in1=t1[:])
                    # k features *= pad (per-partition scalar)
                    k3 = xr_c[:].rearrange("p (h f) -> p h f", f=2 * D)[:, :, D:]
                    nc.vector.tensor_scalar(
                        out=k3,
                        in0=k3,
                        scalar1=padcol[:, c : c + 1],
                        scalar2=None,
                        op0=mult,
                    )
                    xr[c] = xr_c

                # ---- per head: transpose, K=65 live-col matmul, epilogue
                for h in range(H):
                    qkt = qkt_bufs[h % QBUFS]
                    if h < QBUFS:
                        # bias row: [ones(128) | colb chunk] per chunk seg
                        nc.gpsimd.memset(
                            qkt[64:65, :].rearrange(
                                "o (c two f) -> o c two f", two=2, f=128
                            )[:, :, 0, :],
                            1.0,
                        )
                        nc.vector.tensor_copy(
                            out=qkt[64:65, :].rearrange(
                                "o (c two f) -> o c two f", two=2, f=128
                            )[:, :, 1, :],
                            in_=colb[:].rearrange("o (c f) -> o c f", f=128),
                        )
                    for c in reversed(range(MC)):
                        ps_t = pst_pool.tile([64, 256], BF16, tag="pst", bufs=3)
                        nc.tensor.transpose(
                            ps_t[:, 0:128],
                            xr[c][:, h * 2 * D : h * 2 * D + D],
                            ident[:],
                        )
                        nc.tensor.transpose(
                            ps_t[:, 128:256],
                            xr[c][:, h * 2 * D + D : (h + 1) * 2 * D],
                            ident[:],
                        )
                        nc.scalar.copy(
                            out=qkt[0:64, c * 256 : (c + 1) * 256], in_=ps_t[:]
                        )
                    qkt3 = qkt[:].rearrange("p (c two f) -> p c two f", two=2, f=128)
                    osb = osb_bufs[(b * H + h) % OBUFS]
                    early = b == 0 and h < 3
                    last = b == BS - 1 and h == H - 1
                    for c in reversed(range(MC)):
                        w = LW[c]
                        o0 = c * M + c * 128  # live region starts at the diag
                        ps_mm = psm_pool.tile([128, M], F32, tag="psmm", bufs=3)
                        nc.tensor.matmul(
                            ps_mm[:, 0:w],
                            qkt[0:65, c * 256 : c * 256 + 128],
                            qkt3[0:65, c:, 1, :],
                            start=True,
                            stop=True,
                        )
                        # diag block: fused psum + tdiag -> osb (DVE only)
                        nc.vector.tensor_add(
                            out=osb[:, o0 : o0 + 128],
                            in0=ps_mm[:, 0:128],
                            in1=tdiag_sb[:],
                        )
                        if w > 128:
                            # plain live suffix copy, ACT/DVE round-robin
                            use_act = (copy_rr % 4) < 3
                            copy_rr += 1
                            if use_act:
                                nc.scalar.copy(
                                    out=osb[:, o0 + 128 : o0 + w],
                                    in_=ps_mm[:, 128:w],
                                )
                            else:
                                nc.vector.tensor_copy(
                                    out=osb[:, o0 + 128 : o0 + w],
                                    in_=ps_mm[:, 128:w],
                                )
                        if early or last:
                            nc.sync.dma_start(
                                out=out_d[b, h, c * 128 : (c + 1) * 128, :],
                                in_=osb[:, c * M : (c + 1) * M],
                            )
                    if not (early or last):
                        # one full-width 1MB DMA for the whole head
                        nc.sync.dma_start(
                            out=out_d[b, h].rearrange("(c p) n -> p c n", p=128),
                            in_=osb[:].rearrange("p (c n) -> p c n", c=MC),
                        )

    nc.compile()
    return nc


_NC = None
_TABLES = None


def _get_nc():
    global _NC, _TABLES
    if _NC is None:
        _NC = build_nc()
    if _TABLES is None:
        _TABLES = _tables()
    return _NC, _TABLES


def _in_maps(x, attention_mask):
    _, (cos_full, sin_full, tdiag) = _get_nc(), _TABLES
    x = np.ascontiguousarray(np.asarray(x, dtype=np.float32))
    am = np.ascontiguousarray(np.asarray(attention_mask, dtype=np.float32))
    maps = []
    for i in range(N_CORES):
        sl = slice(i * BS, (i + 1) * BS)
        maps.append(
            {
                "x": np.ascontiguousarray(x[sl]),
                "mask": np.ascontiguousarray(am[sl]),
                "cos_t": cos_full,
                "sin_t": sin_full,
                "tdiag": tdiag,
            }
        )
    return maps


def run(x, attention_mask, **run_kwargs):
    nc, _ = _get_nc()
    maps = _in_maps(x, attention_mask)
    res = run_bass_kernel_spmd(nc, maps, list(range(N_CORES)), **run_kwargs)
    out = np.concatenate([r["out"] for r in res.results], axis=0)
    return out, res


def kernel(x, attention_mask, token_type_ids=None, **_unused):
    out, _ = run(x, attention_mask)
    return out
